# revision 7
# baseline (speedup 1.0000x reference)
"""Trainium2 Bass kernel for nn_MultiHeadAttention_38611755991513.

Reference computation (B=2, D=1024, L=2048, H=16, DK=64):
    q/k/v = conv1d(kernel=1) projections of query [B, D, L]
    att   = softmax(mask(q^T k / sqrt(DK)))   with key-only mask [B, 1, L]
    out   = Wo @ (att @ v heads recombined) + bo

Sharding: 32 (batch, head) pairs -> 4 heads (one batch) per core.
Each core computes its 4 heads' attention plus the partial O-projection
(Wo columns for its heads); the host sums the 4 partials per batch.

Key optimization: the mask is key-only, so masked keys are compacted away
on the host (the kernel only ever sees valid keys, zero-padded to a common
length L_c across batches; padded keys get zeroed V rows and a zeroed
ones-column so they contribute nothing to either the attention numerator
or the softmax denominator).

Layout: scores are computed transposed (S^T[k, q]) so that exp(S^T) is
directly the moving operand of the att@v matmul; the softmax denominator
comes for free as a 65th "ones" column of the V operand.
"""

import sys

sys.path.insert(0, "/opt/trn_rl_repo")

import numpy as np
import ml_dtypes

import concourse.bass as bass
import concourse.tile as tile
from concourse import bacc, mybir
from concourse.bass_utils import run_bass_kernel_spmd

B, D, L, H = 2, 1024, 2048, 16
DK = 64
NCORES = 8
HPC = 4              # heads per core
DH = HPC * DK        # 256 head-dims per core
KT = D // 128        # 8 contraction tiles for the projections
BF16 = mybir.dt.bfloat16
F32 = mybir.dt.float32
NPBF16 = ml_dtypes.bfloat16

TRACE = False            # set True (e.g. from test.py) to capture a HW profile
LAST_EXEC_NS = None
LAST_RESULTS = None


def _chunks(total, size):
    out = []
    s = 0
    while s < total:
        w = min(size, total - s)
        out.append((s, w))
        s += w
    return out


def _build(L_c):
    """Build + compile the per-core Bass program for compacted key length L_c."""
    nc = bacc.Bacc("TRN2", debug=False, num_devices=NCORES)
    mts = _chunks(L_c, 128)
    MT = len(mts)
    EXP = mybir.ActivationFunctionType.Exp

    xb_d = nc.declare_dram_parameter("xb", [D, L], BF16, isOutput=False)
    xk_d = nc.declare_dram_parameter("xk", [D, L_c], BF16, isOutput=False)
    vo_d = nc.declare_dram_parameter("vones", [L_c, HPC, 1], F32, isOutput=False)
    wq_d = nc.declare_dram_parameter("wq", [D, DH], BF16, isOutput=False)
    wk_d = nc.declare_dram_parameter("wk", [D, DH], BF16, isOutput=False)
    wv_d = nc.declare_dram_parameter("wv", [D, DH], BF16, isOutput=False)
    wo_d = nc.declare_dram_parameter("wo", [DH, D], BF16, isOutput=False)
    bq_d = nc.declare_dram_parameter("bq", [2, 128, 1], F32, isOutput=False)
    bk_d = nc.declare_dram_parameter("bk", [2, 128, 1], F32, isOutput=False)
    bv_d = nc.declare_dram_parameter("bv", [2, 128, 1], F32, isOutput=False)
    out_d = nc.declare_dram_parameter("out", [D, L], F32, isOutput=True)

    from contextlib import ExitStack
    with tile.TileContext(nc) as tc, ExitStack() as ctx:
        pers = ctx.enter_context(tc.tile_pool(name="pers", bufs=1))

        def ptile(shape, dtype, name):
            return pers.tile(shape, dtype, tag=name, name=name)

        # persistent SBUF tensors
        xk_t = [ptile([128, L_c], BF16, f"xk{i}") for i in range(KT)]
        xb_t = [ptile([128, L], BF16, f"xb{i}") for i in range(KT)]
        wq_t = [ptile([128, DH], BF16, f"wq{i}") for i in range(KT)]
        wk_t = [ptile([128, DH], BF16, f"wk{i}") for i in range(KT)]
        wv_t = [ptile([128, DH], BF16, f"wv{i}") for i in range(KT)]
        wo_t = [ptile([128, D], BF16, f"wo{i}") for i in range(2)]
        bq_t = [ptile([128, 1], F32, f"bq{i}") for i in range(2)]
        bk_t = [ptile([128, 1], F32, f"bk{i}") for i in range(2)]
        bv_t = [ptile([128, 1], F32, f"bv{i}") for i in range(2)]
        vo_t = [ptile([mw, HPC, 1], F32, f"vo{mt}") for mt, (ms, mw) in enumerate(mts)]
        q_t = [ptile([128, L], BF16, f"q{i}") for i in range(2)]
        k_t = [ptile([128, L_c], BF16, f"k{i}") for i in range(2)]
        z_t = [ptile([128, L], BF16, f"z{i}") for i in range(2)]
        va_t = [ptile([mw, HPC, 65], BF16, f"va{mt}") for mt, (ms, mw) in enumerate(mts)]

        # input DMAs (K-projection operands first so scores can start early)
        for i in range(KT):
            nc.sync.dma_start(xk_t[i][:], xk_d[i * 128:(i + 1) * 128, :])
            nc.sync.dma_start(wk_t[i][:], wk_d[i * 128:(i + 1) * 128, :])
        for i in range(2):
            nc.sync.dma_start(bk_t[i][:], bk_d[i])
            nc.sync.dma_start(bq_t[i][:], bq_d[i])
            nc.sync.dma_start(bv_t[i][:], bv_d[i])
        for i in range(KT):
            nc.sync.dma_start(xb_t[i][:], xb_d[i * 128:(i + 1) * 128, :])
            nc.sync.dma_start(wq_t[i][:], wq_d[i * 128:(i + 1) * 128, :])
            nc.sync.dma_start(wv_t[i][:], wv_d[i * 128:(i + 1) * 128, :])
        for mt, (ms, mw) in enumerate(mts):
            nc.sync.dma_start(vo_t[mt][:], vo_d[ms:ms + mw])
        for i in range(2):
            nc.sync.dma_start(wo_t[i][:], wo_d[i * 128:(i + 1) * 128, :])

        with (
            tc.tile_pool(name="psA", bufs=2, space="PSUM") as pa,
            tc.tile_pool(name="psB", bufs=2, space="PSUM") as pb,
            tc.tile_pool(name="psR", bufs=1, space="PSUM") as pr,
            tc.tile_pool(name="pexp", bufs=3 * MT) as pp,
            tc.tile_pool(name="osb", bufs=3) as po,
            tc.tile_pool(name="small", bufs=3) as psm,
        ):
            # ---- K projection: k = (Wk x_k) + bk, laid out [d, k_pos] ----
            for kt in range(2):
                for ns, nw in _chunks(L_c, 1024):
                    kp = pa.tile([128, nw], F32, tag="wide", name=f"kp{kt}_{ns}")
                    for kk in range(KT):
                        for js, jw in _chunks(nw, 512):
                            nc.tensor.matmul(
                                kp[:, js:js + jw],
                                wk_t[kk][:, kt * 128:(kt + 1) * 128],
                                xk_t[kk][:, ns + js:ns + js + jw],
                                start=(kk == 0), stop=(kk == KT - 1),
                            )
                    nc.vector.tensor_scalar_add(k_t[kt][:, ns:ns + nw], kp[:], bk_t[kt][:])

            # ---- Q projection (scale 1/sqrt(DK) folded into wq/bq) ----
            for kt in range(2):
                for ns, nw in _chunks(L, 1024):
                    qp = pa.tile([128, nw], F32, tag="wide", name=f"qp{kt}_{ns}")
                    for kk in range(KT):
                        for js, jw in _chunks(nw, 512):
                            nc.tensor.matmul(
                                qp[:, js:js + jw],
                                wq_t[kk][:, kt * 128:(kt + 1) * 128],
                                xb_t[kk][:, ns + js:ns + js + jw],
                                start=(kk == 0), stop=(kk == KT - 1),
                            )
                    nc.vector.tensor_scalar_add(q_t[kt][:, ns:ns + nw], qp[:], bq_t[kt][:])

            # ---- V^T (no bias; bias folded in post-normalize) + ones column ----
            for mt, (ms, mw) in enumerate(mts):
                vp = pa.tile([mw, DH], F32, tag="wide", name=f"vp{mt}")
                for kk in range(KT):
                    nc.tensor.matmul(
                        vp[:],
                        xk_t[kk][:, ms:ms + mw],
                        wv_t[kk][:],
                        start=(kk == 0), stop=(kk == KT - 1),
                    )
                for h in range(HPC):
                    nc.vector.tensor_copy(va_t[mt][:, h, 0:64], vp[:, h * 64:(h + 1) * 64])
                nc.vector.tensor_copy(va_t[mt][:, :, 64:65], vo_t[mt][:])

            # ---- attention + partial O-projection, per 1024-wide q block ----
            # Heads are processed in pairs (one 128-partition ptile): the two
            # heads' score matmuls row-pack the PE array (K=64 at array rows
            # 0-63 / 64-127, concurrent), and their att@v matmuls col-pack it
            # (M=64 at output partitions 0-63 / 64-127). Softmax denominators
            # come from packed M=1 matmuls against the va ones-column.
            for qs, qw in _chunks(L, 1024):
                for hp in range(2):
                    offs = (0, 64)
                    pA, pB = [], []
                    for mt, (ms, mw) in enumerate(mts):
                        sps = []
                        for hi, off in enumerate(offs):
                            sp = pa.tile([mw, qw], F32, tag="wide", name=f"s{qs}_{hp}_{mt}_{hi}")
                            for js, jw in _chunks(qw, 512):
                                nc.tensor.matmul(
                                    sp[:, js:js + jw],
                                    k_t[hp][off:off + 64, ms:ms + mw],
                                    q_t[hp][off:off + 64, qs + js:qs + js + jw],
                                    start=True, stop=True,
                                )
                            sps.append(sp)
                        for hi, sp in enumerate(sps):
                            px = pp.tile([mw, qw], BF16, tag="p", name=f"p{qs}_{hp}_{mt}_{hi}")
                            nc.scalar.activation(px[:], sp[:], EXP)
                            (pA, pB)[hi].append(px)
                    hA, hB = 2 * hp, 2 * hp + 1
                    for js, jw in _chunks(qw, 512):
                        yp = pb.tile([128, jw], F32, tag="y", name=f"y{qs}_{hp}_{js}")
                        rpA = pr.tile([1, jw], F32, tag="rA", name=f"rA{qs}_{hp}_{js}")
                        rpB = pr.tile([1, jw], F32, tag="rB", name=f"rB{qs}_{hp}_{js}")
                        for mt, (ms, mw) in enumerate(mts):
                            st, sp_ = (mt == 0), (mt == MT - 1)
                            nc.tensor.matmul(
                                yp[0:64, :], va_t[mt][:, hA, 0:64],
                                pA[mt][:, js:js + jw], start=st, stop=sp_,
                                skip_group_check=True,
                            )
                            nc.tensor.matmul(
                                yp[64:128, :], va_t[mt][:, hB, 0:64],
                                pB[mt][:, js:js + jw], start=st, stop=sp_,
                                skip_group_check=True,
                            )
                            nc.tensor.matmul(
                                rpA[:], va_t[mt][:, hA, 64:65],
                                pA[mt][:, js:js + jw], start=st, stop=sp_,
                            )
                            nc.tensor.matmul(
                                rpB[:], va_t[mt][:, hB, 64:65],
                                pB[mt][:, js:js + jw], start=st, stop=sp_,
                            )
                        rb = psm.tile([128, jw], F32, tag="rb", name=f"rb{qs}_{hp}_{js}")
                        rcA = psm.tile([1, jw], F32, tag="rcA", name=f"rcA{qs}_{hp}_{js}")
                        rcB = psm.tile([1, jw], F32, tag="rcB", name=f"rcB{qs}_{hp}_{js}")
                        rbB = psm.tile([64, jw], F32, tag="rbB", name=f"rbB{qs}_{hp}_{js}")
                        nc.vector.reciprocal_approx_fast(rcA[:], rpA[:])
                        nc.vector.reciprocal_approx_fast(rcB[:], rpB[:])
                        nc.gpsimd.partition_broadcast(rb[0:64, :], rcA[:])
                        nc.gpsimd.partition_broadcast(rbB[:], rcB[:])
                        # partition_broadcast can't target base-64 partitions on
                        # HW; shift the second half up with a DVE copy instead.
                        nc.vector.tensor_copy(rb[64:128, :], rbB[:])
                        zsl = z_t[hp][:, qs + js:qs + js + jw]
                        nc.vector.tensor_mul(zsl, yp[:], rb[:])
                        nc.vector.tensor_scalar_add(zsl, zsl, bv_t[hp][:])
                # partial O-projection for this q block (needs all 4 heads' z)
                for m8 in range(8):
                    op = pa.tile([128, qw], F32, tag="wide", name=f"o{qs}_{m8}")
                    for kt in range(2):
                        for js, jw in _chunks(qw, 512):
                            nc.tensor.matmul(
                                op[:, js:js + jw],
                                wo_t[kt][:, m8 * 128:(m8 + 1) * 128],
                                z_t[kt][:, qs + js:qs + js + jw],
                                start=(kt == 0), stop=(kt == 1),
                            )
                    ob = po.tile([128, qw], F32, tag="ob", name=f"ob{qs}_{m8}")
                    nc.vector.tensor_copy(ob[:], op[:])
                    nc.sync.dma_start(out_d[m8 * 128:(m8 + 1) * 128, qs:qs + qw], ob[:])

    nc.compile()
    return nc


_NC_CACHE = {}


def _get_nc(L_c):
    if L_c not in _NC_CACHE:
        _NC_CACHE[L_c] = _build(L_c)
    return _NC_CACHE[L_c]


def _install_ntff_hook():
    """Synthesize antenv.axon_hooks (missing in this image) so trace=True works."""
    import types

    if "antenv.axon_hooks" in sys.modules:
        return
    try:
        if "/root/.axon_site" not in sys.path:
            sys.path.insert(0, "/root/.axon_site")
        from trn_agent_boot.trn_boot import _ntff_profile_via_ctypes

        hook = _ntff_profile_via_ctypes("/opt/axon/libaxon_pjrt.so")
        mod = types.ModuleType("antenv.axon_hooks")
        mod.get_axon_ntff_profile_hook = lambda: hook
        import antenv  # noqa: F401

        sys.modules["antenv.axon_hooks"] = mod
    except Exception:
        pass


def kernel(query, att_mask, Wq, bq, Wk, bk, Wv, bv, Wo, bo):
    global LAST_EXEC_NS, LAST_RESULTS
    query = np.asarray(query, dtype=np.float32)
    mask = np.asarray(att_mask).astype(bool).reshape(B, L)
    Wq, bq = np.asarray(Wq, np.float32), np.asarray(bq, np.float32)
    Wk, bk = np.asarray(Wk, np.float32), np.asarray(bk, np.float32)
    Wv, bv = np.asarray(Wv, np.float32), np.asarray(bv, np.float32)
    Wo, bo = np.asarray(Wo, np.float32), np.asarray(bo, np.float32)

    valid = [np.nonzero(mask[b])[0] for b in range(B)]
    L_c = max(len(v) for v in valid)
    out = np.empty((B, D, L), np.float32)
    if L_c == 0:
        out[:] = bo[None, :, None]
        return out

    scale = np.float32(1.0 / np.sqrt(DK))
    # per-batch compacted keys + validity column
    xk_b, vones_b, xb_b = [], [], []
    for b in range(B):
        idx = valid[b]
        xk = np.zeros((D, L_c), np.float32)
        xk[:, :len(idx)] = query[b][:, idx]
        xk_b.append(xk.astype(NPBF16))
        vo = np.zeros((L_c, HPC, 1), np.float32)
        vo[:len(idx)] = 1.0
        vones_b.append(vo)
        xb_b.append(query[b].astype(NPBF16))

    in_maps = []
    for c in range(NCORES):
        b, g = divmod(c, NCORES // B)
        sl = slice(g * DH, (g + 1) * DH)
        in_maps.append({
            "xb": xb_b[b],
            "xk": xk_b[b],
            "vones": vones_b[b],
            "wq": np.ascontiguousarray((Wq[sl, :] * scale).T).astype(NPBF16),
            "wk": np.ascontiguousarray(Wk[sl, :].T).astype(NPBF16),
            "wv": np.ascontiguousarray(Wv[sl, :].T).astype(NPBF16),
            "wo": np.ascontiguousarray(Wo[:, sl].T).astype(NPBF16),
            "bq": (bq[sl] * scale).reshape(2, 128, 1).astype(np.float32),
            "bk": bk[sl].reshape(2, 128, 1).astype(np.float32),
            "bv": bv[sl].reshape(2, 128, 1).astype(np.float32),
        })

    nc = _get_nc(L_c)
    if TRACE:
        _install_ntff_hook()
    res = run_bass_kernel_spmd(nc, in_maps, core_ids=list(range(NCORES)), trace=TRACE)
    LAST_EXEC_NS = res.exec_time_ns
    LAST_RESULTS = res

    parts = [res.results[c]["out"] for c in range(NCORES)]
    for b in range(B):
        if len(valid[b]) == 0:
            out[b] = bo[:, None]
        else:
            acc = parts[4 * b].astype(np.float32)
            for g in range(1, 4):
                acc = acc + parts[4 * b + g]
            out[b] = acc + bo[:, None]
    return out


# revision 9
# speedup vs baseline: 1.1779x; 1.1779x over previous
"""Trainium2 Bass kernel for nn_MultiHeadAttention_38611755991513.

Reference computation (B=2, D=1024, L=2048, H=16, DK=64):
    q/k/v = conv1d(kernel=1) projections of query [B, D, L]
    att   = softmax(mask(q^T k / sqrt(DK)))   with key-only mask [B, 1, L]
    out   = Wo @ (att @ v heads recombined) + bo

Sharding: 32 (batch, head) pairs -> 4 heads (one batch) per core.
Each core computes its 4 heads' attention plus the partial O-projection
(Wo columns for its heads); the host sums the 4 partials per batch.

Key optimization: the mask is key-only, so masked keys are compacted away
on the host (the kernel only ever sees valid keys, zero-padded to a common
length L_c across batches; padded keys get zeroed V rows and a zeroed
ones-column so they contribute nothing to either the attention numerator
or the softmax denominator).

Layout: scores are computed transposed (S^T[k, q]) so that exp(S^T) is
directly the moving operand of the att@v matmul; the softmax denominator
comes for free as a 65th "ones" column of the V operand.
"""

import sys

sys.path.insert(0, "/opt/trn_rl_repo")

import numpy as np
import ml_dtypes

import concourse.bass as bass
import concourse.tile as tile
from concourse import bacc, mybir
from concourse.bass_utils import run_bass_kernel_spmd

B, D, L, H = 2, 1024, 2048, 16
DK = 64
NCORES = 8
HPC = 4              # heads per core
DH = HPC * DK        # 256 head-dims per core
KT = D // 128        # 8 contraction tiles for the projections
BF16 = mybir.dt.bfloat16
F32 = mybir.dt.float32
NPBF16 = ml_dtypes.bfloat16

TRACE = False            # set True (e.g. from test.py) to capture a HW profile
LAST_EXEC_NS = None
LAST_RESULTS = None


def _chunks(total, size):
    out = []
    s = 0
    while s < total:
        w = min(size, total - s)
        out.append((s, w))
        s += w
    return out


def _build(L_c):
    """Build + compile the per-core Bass program for compacted key length L_c."""
    nc = bacc.Bacc("TRN2", debug=False, num_devices=NCORES)
    mts = _chunks(L_c, 128)
    MT = len(mts)
    EXP = mybir.ActivationFunctionType.Exp

    xb_d = nc.declare_dram_parameter("xb", [D, L], BF16, isOutput=False)
    xk_d = nc.declare_dram_parameter("xk", [D, L_c], BF16, isOutput=False)
    vo_d = nc.declare_dram_parameter("vones", [L_c, HPC, 1], F32, isOutput=False)
    wq_d = nc.declare_dram_parameter("wq", [D, DH], BF16, isOutput=False)
    wk_d = nc.declare_dram_parameter("wk", [D, DH], BF16, isOutput=False)
    wv_d = nc.declare_dram_parameter("wv", [D, DH], BF16, isOutput=False)
    wo_d = nc.declare_dram_parameter("wo", [DH, D], BF16, isOutput=False)
    bq_d = nc.declare_dram_parameter("bq", [2, 128, 1], F32, isOutput=False)
    bk_d = nc.declare_dram_parameter("bk", [2, 128, 1], F32, isOutput=False)
    bv_d = nc.declare_dram_parameter("bv", [2, 128, 1], F32, isOutput=False)
    out_d = nc.declare_dram_parameter("out", [D, L], F32, isOutput=True)

    from contextlib import ExitStack
    with tile.TileContext(nc) as tc, ExitStack() as ctx:
        pers = ctx.enter_context(tc.tile_pool(name="pers", bufs=1))

        def ptile(shape, dtype, name):
            return pers.tile(shape, dtype, tag=name, name=name)

        # persistent SBUF tensors
        xk_t = [ptile([128, L_c], BF16, f"xk{i}") for i in range(KT)]
        xb_t = [ptile([128, L], BF16, f"xb{i}") for i in range(KT)]
        wq_t = [ptile([128, DH], BF16, f"wq{i}") for i in range(KT)]
        wk_t = [ptile([128, DH], BF16, f"wk{i}") for i in range(KT)]
        wv_t = [ptile([128, DH], BF16, f"wv{i}") for i in range(KT)]
        wo_t = [ptile([128, D], BF16, f"wo{i}") for i in range(2)]
        bq_t = [ptile([128, 1], F32, f"bq{i}") for i in range(2)]
        bk_t = [ptile([128, 1], F32, f"bk{i}") for i in range(2)]
        bv_t = [ptile([128, 1], F32, f"bv{i}") for i in range(2)]
        vo_t = [ptile([mw, HPC, 1], F32, f"vo{mt}") for mt, (ms, mw) in enumerate(mts)]
        q_t = [ptile([128, L], BF16, f"q{i}") for i in range(2)]
        k_t = [ptile([128, L_c], BF16, f"k{i}") for i in range(2)]
        z_t = [ptile([128, L], BF16, f"z{i}") for i in range(2)]
        va_t = [ptile([mw, HPC, 65], BF16, f"va{mt}") for mt, (ms, mw) in enumerate(mts)]

        # input DMAs (K-projection operands first so scores can start early)
        for i in range(KT):
            nc.sync.dma_start(xk_t[i][:], xk_d[i * 128:(i + 1) * 128, :])
            nc.sync.dma_start(wk_t[i][:], wk_d[i * 128:(i + 1) * 128, :])
        for i in range(2):
            nc.sync.dma_start(bk_t[i][:], bk_d[i])
            nc.sync.dma_start(bq_t[i][:], bq_d[i])
            nc.sync.dma_start(bv_t[i][:], bv_d[i])
        for i in range(KT):
            nc.sync.dma_start(xb_t[i][:], xb_d[i * 128:(i + 1) * 128, :])
            nc.sync.dma_start(wq_t[i][:], wq_d[i * 128:(i + 1) * 128, :])
            nc.sync.dma_start(wv_t[i][:], wv_d[i * 128:(i + 1) * 128, :])
        for mt, (ms, mw) in enumerate(mts):
            nc.sync.dma_start(vo_t[mt][:], vo_d[ms:ms + mw])
        for i in range(2):
            nc.sync.dma_start(wo_t[i][:], wo_d[i * 128:(i + 1) * 128, :])

        with (
            tc.tile_pool(name="psA", bufs=2, space="PSUM") as pa,
            tc.tile_pool(name="psY", bufs=2, space="PSUM") as pb,
            tc.tile_pool(name="psO", bufs=2, space="PSUM") as pox,
            tc.tile_pool(name="pexp", bufs=3 * MT) as pp,
            tc.tile_pool(name="osb", bufs=3) as po,
            tc.tile_pool(name="small", bufs=3) as psm,
        ):
            # ---- K projection: k = (Wk x_k) + bk, laid out [d, k_pos] ----
            for kt in range(2):
                for ns, nw in _chunks(L_c, 512):
                    kp = pox.tile([128, nw], F32, tag="po", name=f"kp{kt}_{ns}")
                    for kk in range(KT):
                        nc.tensor.matmul(
                            kp[:],
                            wk_t[kk][:, kt * 128:(kt + 1) * 128],
                            xk_t[kk][:, ns:ns + nw],
                            start=(kk == 0), stop=(kk == KT - 1),
                        )
                    nc.vector.tensor_scalar_add(k_t[kt][:, ns:ns + nw], kp[:], bk_t[kt][:])

            def q_proj_block(qs, qw):
                for kt in range(2):
                    for js, jw in _chunks(qw, 512):
                        qp = pox.tile([128, jw], F32, tag="po", name=f"qp{kt}_{qs + js}")
                        for kk in range(KT):
                            nc.tensor.matmul(
                                qp[:],
                                wq_t[kk][:, kt * 128:(kt + 1) * 128],
                                xb_t[kk][:, qs + js:qs + js + jw],
                                start=(kk == 0), stop=(kk == KT - 1),
                            )
                        nc.vector.tensor_scalar_add(q_t[kt][:, qs + js:qs + js + jw], qp[:], bq_t[kt][:])

            # ---- Q projection for the first q block ----
            qblocks = _chunks(L, 1024)
            q_proj_block(*qblocks[0])

            # ---- V^T (no bias; bias folded in post-normalize) + ones column ----
            for mt, (ms, mw) in enumerate(mts):
                vp = pox.tile([mw, DH], F32, tag="po", name=f"vp{mt}")
                for kk in range(KT):
                    nc.tensor.matmul(
                        vp[:],
                        xk_t[kk][:, ms:ms + mw],
                        wv_t[kk][:],
                        start=(kk == 0), stop=(kk == KT - 1),
                    )
                for h in range(HPC):
                    nc.vector.tensor_copy(va_t[mt][:, h, 0:64], vp[:, h * 64:(h + 1) * 64])
                nc.vector.tensor_copy(va_t[mt][:, :, 64:65], vo_t[mt][:])

            # ---- attention + partial O-projection, per 1024-wide q block ----
            for qi, (qs, qw) in enumerate(qblocks):
                for h in range(HPC):
                    pt, off = h // 2, (h % 2) * 64
                    p_tiles = []
                    for mt, (ms, mw) in enumerate(mts):
                        sp = pa.tile([mw, qw], F32, tag="wide", name=f"s{qs}_{h}_{mt}")
                        for js, jw in _chunks(qw, 512):
                            nc.tensor.matmul(
                                sp[:, js:js + jw],
                                k_t[pt][off:off + 64, ms:ms + mw],
                                q_t[pt][off:off + 64, qs + js:qs + js + jw],
                                start=True, stop=True,
                            )
                        px = pp.tile([mw, qw], BF16, tag="p", name=f"p{qs}_{h}_{mt}")
                        nc.scalar.activation(px[:], sp[:], EXP)
                        p_tiles.append(px)
                    for js, jw in _chunks(qw, 512):
                        yp = pb.tile([65, jw], F32, tag="y", name=f"y{qs}_{h}_{js}")
                        for mt, (ms, mw) in enumerate(mts):
                            nc.tensor.matmul(
                                yp[:],
                                va_t[mt][:, h, :],
                                p_tiles[mt][:, js:js + jw],
                                start=(mt == 0), stop=(mt == MT - 1),
                            )
                        rt = psm.tile([1, jw], F32, tag="rrow", name=f"rt{qs}_{h}_{js}")
                        nc.vector.tensor_copy(rt[:], yp[64:65, :])
                        rc = psm.tile([1, jw], F32, tag="recip", name=f"rc{qs}_{h}_{js}")
                        nc.vector.reciprocal_approx_fast(rc[:], rt[:])
                        rb = psm.tile([64, jw], F32, tag="rb", name=f"rb{qs}_{h}_{js}")
                        nc.gpsimd.partition_broadcast(rb[:], rc[:])
                        zsl = z_t[pt][off:off + 64, qs + js:qs + js + jw]
                        nc.vector.tensor_mul(zsl, yp[0:64, :], rb[:])
                        nc.vector.tensor_scalar_add(zsl, zsl, bv_t[pt][off:off + 64, :])
                    if h == 0 and qi + 1 < len(qblocks):
                        # Q-projection for the next block: PE filler work that
                        # overlaps this block's ACT-bound exp stream.
                        q_proj_block(*qblocks[qi + 1])
                # partial O-projection for this q block (needs all 4 heads' z)
                for m8 in range(8):
                    for js, jw in _chunks(qw, 512):
                        op = pox.tile([128, jw], F32, tag="po", name=f"o{qs}_{m8}_{js}")
                        for kt in range(2):
                            nc.tensor.matmul(
                                op[:],
                                wo_t[kt][:, m8 * 128:(m8 + 1) * 128],
                                z_t[kt][:, qs + js:qs + js + jw],
                                start=(kt == 0), stop=(kt == 1),
                            )
                        ob = po.tile([128, jw], F32, tag="ob", name=f"ob{qs}_{m8}_{js}")
                        nc.vector.tensor_copy(ob[:], op[:])
                        nc.sync.dma_start(out_d[m8 * 128:(m8 + 1) * 128, qs + js:qs + js + jw], ob[:])

    nc.compile()
    return nc


_NC_CACHE = {}


def _get_nc(L_c):
    if L_c not in _NC_CACHE:
        _NC_CACHE[L_c] = _build(L_c)
    return _NC_CACHE[L_c]


def _install_ntff_hook():
    """Synthesize antenv.axon_hooks (missing in this image) so trace=True works."""
    import types

    if "antenv.axon_hooks" in sys.modules:
        return
    try:
        if "/root/.axon_site" not in sys.path:
            sys.path.insert(0, "/root/.axon_site")
        from trn_agent_boot.trn_boot import _ntff_profile_via_ctypes

        hook = _ntff_profile_via_ctypes("/opt/axon/libaxon_pjrt.so")
        mod = types.ModuleType("antenv.axon_hooks")
        mod.get_axon_ntff_profile_hook = lambda: hook
        import antenv  # noqa: F401

        sys.modules["antenv.axon_hooks"] = mod
    except Exception:
        pass


def kernel(query, att_mask, Wq, bq, Wk, bk, Wv, bv, Wo, bo):
    global LAST_EXEC_NS, LAST_RESULTS
    query = np.asarray(query, dtype=np.float32)
    mask = np.asarray(att_mask).astype(bool).reshape(B, L)
    Wq, bq = np.asarray(Wq, np.float32), np.asarray(bq, np.float32)
    Wk, bk = np.asarray(Wk, np.float32), np.asarray(bk, np.float32)
    Wv, bv = np.asarray(Wv, np.float32), np.asarray(bv, np.float32)
    Wo, bo = np.asarray(Wo, np.float32), np.asarray(bo, np.float32)

    valid = [np.nonzero(mask[b])[0] for b in range(B)]
    L_c = max(len(v) for v in valid)
    out = np.empty((B, D, L), np.float32)
    if L_c == 0:
        out[:] = bo[None, :, None]
        return out

    scale = np.float32(1.0 / np.sqrt(DK))
    # per-batch compacted keys + validity column
    xk_b, vones_b, xb_b = [], [], []
    for b in range(B):
        idx = valid[b]
        xk = np.zeros((D, L_c), np.float32)
        xk[:, :len(idx)] = query[b][:, idx]
        xk_b.append(xk.astype(NPBF16))
        vo = np.zeros((L_c, HPC, 1), np.float32)
        vo[:len(idx)] = 1.0
        vones_b.append(vo)
        xb_b.append(query[b].astype(NPBF16))

    in_maps = []
    for c in range(NCORES):
        b, g = divmod(c, NCORES // B)
        sl = slice(g * DH, (g + 1) * DH)
        in_maps.append({
            "xb": xb_b[b],
            "xk": xk_b[b],
            "vones": vones_b[b],
            "wq": np.ascontiguousarray((Wq[sl, :] * scale).T).astype(NPBF16),
            "wk": np.ascontiguousarray(Wk[sl, :].T).astype(NPBF16),
            "wv": np.ascontiguousarray(Wv[sl, :].T).astype(NPBF16),
            "wo": np.ascontiguousarray(Wo[:, sl].T).astype(NPBF16),
            "bq": (bq[sl] * scale).reshape(2, 128, 1).astype(np.float32),
            "bk": bk[sl].reshape(2, 128, 1).astype(np.float32),
            "bv": bv[sl].reshape(2, 128, 1).astype(np.float32),
        })

    nc = _get_nc(L_c)
    if TRACE:
        _install_ntff_hook()
    res = run_bass_kernel_spmd(nc, in_maps, core_ids=list(range(NCORES)), trace=TRACE)
    LAST_EXEC_NS = res.exec_time_ns
    LAST_RESULTS = res

    parts = [res.results[c]["out"] for c in range(NCORES)]
    for b in range(B):
        if len(valid[b]) == 0:
            out[b] = bo[:, None]
        else:
            acc = parts[4 * b].astype(np.float32)
            for g in range(1, 4):
                acc = acc + parts[4 * b + g]
            out[b] = acc + bo[:, None]
    return out


# revision 12
# speedup vs baseline: 1.2721x; 1.0800x over previous
"""Trainium2 Bass kernel for nn_MultiHeadAttention_38611755991513.

Reference computation (B=2, D=1024, L=2048, H=16, DK=64):
    q/k/v = conv1d(kernel=1) projections of query [B, D, L]
    att   = softmax(mask(q^T k / sqrt(DK)))   with key-only mask [B, 1, L]
    out   = Wo @ (att @ v heads recombined) + bo

Sharding: 32 (batch, head) pairs -> 4 heads (one batch) per core.
Each core computes its 4 heads' attention plus the partial O-projection
(Wo columns for its heads); the host sums the 4 partials per batch.

Key optimization: the mask is key-only, so masked keys are compacted away
on the host (the kernel only ever sees valid keys, zero-padded to a common
length L_c across batches; padded keys get zeroed V rows and a zeroed
ones-column so they contribute nothing to either the attention numerator
or the softmax denominator).

Layout: scores are computed transposed (S^T[k, q]) so that exp(S^T) is
directly the moving operand of the att@v matmul; the softmax denominator
comes for free as a 65th "ones" column of the V operand.
"""

import sys

sys.path.insert(0, "/opt/trn_rl_repo")

import numpy as np
import ml_dtypes

import concourse.bass as bass
import concourse.tile as tile
from concourse import bacc, mybir
from concourse.bass_utils import run_bass_kernel_spmd

B, D, L, H = 2, 1024, 2048, 16
DK = 64
NCORES = 8
HPC = 4              # heads per core
DH = HPC * DK        # 256 head-dims per core
KT = D // 128        # 8 contraction tiles for the projections
BF16 = mybir.dt.bfloat16
F32 = mybir.dt.float32
NPBF16 = ml_dtypes.bfloat16

TRACE = False            # set True (e.g. from test.py) to capture a HW profile
LAST_EXEC_NS = None
LAST_RESULTS = None


def _chunks(total, size):
    out = []
    s = 0
    while s < total:
        w = min(size, total - s)
        out.append((s, w))
        s += w
    return out


def _build(L_c):
    """Build + compile the per-core Bass program for compacted key length L_c."""
    nc = bacc.Bacc("TRN2", debug=False, num_devices=NCORES)
    mts = _chunks(L_c, 128)
    MT = len(mts)
    EXP = mybir.ActivationFunctionType.Exp

    xb_d = nc.declare_dram_parameter("xb", [D, L], BF16, isOutput=False)
    xk_d = nc.declare_dram_parameter("xk", [D, L_c], BF16, isOutput=False)
    vo_d = nc.declare_dram_parameter("vones", [L_c, HPC, 1], F32, isOutput=False)
    wq_d = nc.declare_dram_parameter("wq", [D, DH], BF16, isOutput=False)
    wk_d = nc.declare_dram_parameter("wk", [D, DH], BF16, isOutput=False)
    wv_d = nc.declare_dram_parameter("wv", [D, DH], BF16, isOutput=False)
    wo_d = nc.declare_dram_parameter("wo", [DH, D], BF16, isOutput=False)
    bq_d = nc.declare_dram_parameter("bq", [2, 128, 1], F32, isOutput=False)
    bk_d = nc.declare_dram_parameter("bk", [2, 128, 1], F32, isOutput=False)
    bv_d = nc.declare_dram_parameter("bv", [2, 128, 1], F32, isOutput=False)
    out_d = nc.declare_dram_parameter("out", [D, L], F32, isOutput=True)

    from contextlib import ExitStack
    with tile.TileContext(nc) as tc, ExitStack() as ctx:
        pers = ctx.enter_context(tc.tile_pool(name="pers", bufs=1))

        def ptile(shape, dtype, name):
            return pers.tile(shape, dtype, tag=name, name=name)

        # persistent SBUF tensors
        xk_t = [ptile([128, L_c], BF16, f"xk{i}") for i in range(KT)]
        xb_t = [ptile([128, L], BF16, f"xb{i}") for i in range(KT)]
        wq_t = [ptile([128, DH], BF16, f"wq{i}") for i in range(KT)]
        wk_t = [ptile([128, DH], BF16, f"wk{i}") for i in range(KT)]
        wv_t = [ptile([128, DH], BF16, f"wv{i}") for i in range(KT)]
        wo_t = [ptile([128, D], BF16, f"wo{i}") for i in range(2)]
        bq_t = [ptile([128, 1], F32, f"bq{i}") for i in range(2)]
        bk_t = [ptile([128, 1], F32, f"bk{i}") for i in range(2)]
        bv_t = [ptile([128, 1], F32, f"bv{i}") for i in range(2)]
        vo_t = [ptile([mw, HPC, 1], F32, f"vo{mt}") for mt, (ms, mw) in enumerate(mts)]
        q_t = [ptile([128, L], BF16, f"q{i}") for i in range(2)]
        k_t = [ptile([128, L_c], BF16, f"k{i}") for i in range(2)]
        z_t = [ptile([128, L], BF16, f"z{i}") for i in range(2)]
        va_t = [ptile([mw, HPC, 65], BF16, f"va{mt}") for mt, (ms, mw) in enumerate(mts)]

        # input DMAs (K-projection operands first so scores can start early)
        for i in range(KT):
            nc.sync.dma_start(xk_t[i][:], xk_d[i * 128:(i + 1) * 128, :])
            nc.sync.dma_start(wk_t[i][:], wk_d[i * 128:(i + 1) * 128, :])
        for i in range(2):
            nc.sync.dma_start(bk_t[i][:], bk_d[i])
            nc.sync.dma_start(bq_t[i][:], bq_d[i])
            nc.sync.dma_start(bv_t[i][:], bv_d[i])
        for i in range(KT):
            nc.sync.dma_start(xb_t[i][:], xb_d[i * 128:(i + 1) * 128, :])
            nc.sync.dma_start(wq_t[i][:], wq_d[i * 128:(i + 1) * 128, :])
            nc.sync.dma_start(wv_t[i][:], wv_d[i * 128:(i + 1) * 128, :])
        for mt, (ms, mw) in enumerate(mts):
            nc.sync.dma_start(vo_t[mt][:], vo_d[ms:ms + mw])
        for i in range(2):
            nc.sync.dma_start(wo_t[i][:], wo_d[i * 128:(i + 1) * 128, :])

        with (
            tc.tile_pool(name="psA", bufs=2, space="PSUM") as pa,
            tc.tile_pool(name="psY", bufs=2, space="PSUM") as pb,
            tc.tile_pool(name="psO", bufs=2, space="PSUM") as pox,
            tc.tile_pool(name="pexp", bufs=3 * MT) as pp,
            tc.tile_pool(name="osb", bufs=3) as po,
            tc.tile_pool(name="small", bufs=3) as psm,
        ):
            # ---- K projection: k = (Wk x_k) + bk, laid out [d, k_pos] ----
            for kt in range(2):
                for ns, nw in _chunks(L_c, 512):
                    kp = pox.tile([128, nw], F32, tag="po", name=f"kp{kt}_{ns}")
                    for kk in range(KT):
                        nc.tensor.matmul(
                            kp[:],
                            wk_t[kk][:, kt * 128:(kt + 1) * 128],
                            xk_t[kk][:, ns:ns + nw],
                            start=(kk == 0), stop=(kk == KT - 1),
                        )
                    nc.vector.tensor_scalar_add(k_t[kt][:, ns:ns + nw], kp[:], bk_t[kt][:])

            qblocks = _chunks(L, 1024)

            def q_chain(qs, kt, js, jw):
                qp = pox.tile([128, jw], F32, tag="po", name=f"qp{kt}_{qs + js}")
                for kk in range(KT):
                    nc.tensor.matmul(
                        qp[:],
                        wq_t[kk][:, kt * 128:(kt + 1) * 128],
                        xb_t[kk][:, qs + js:qs + js + jw],
                        start=(kk == 0), stop=(kk == KT - 1),
                    )
                nc.vector.tensor_scalar_add(q_t[kt][:, qs + js:qs + js + jw], qp[:], bq_t[kt][:])

            # Q projection for the first q block (scale 1/sqrt(DK) folded in wq/bq)
            for kt in range(2):
                for js, jw in _chunks(1024, 512):
                    q_chain(0, kt, js, jw)

            def v_chain(mt):
                ms, mw = mts[mt]
                vp = pox.tile([mw, DH], F32, tag="po", name=f"vp{mt}")
                for kk in range(KT):
                    nc.tensor.matmul(
                        vp[:],
                        xk_t[kk][:, ms:ms + mw],
                        wv_t[kk][:],
                        start=(kk == 0), stop=(kk == KT - 1),
                    )
                for h in range(HPC):
                    nc.vector.tensor_copy(va_t[mt][:, h, 0:64], vp[:, h * 64:(h + 1) * 64])
                nc.vector.tensor_copy(va_t[mt][:, :, 64:65], vo_t[mt][:])

            def o_chunk(qs, m8, js, jw):
                op = pox.tile([128, jw], F32, tag="po", name=f"o{qs}_{m8}_{js}")
                for kt in range(2):
                    nc.tensor.matmul(
                        op[:],
                        wo_t[kt][:, m8 * 128:(m8 + 1) * 128],
                        z_t[kt][:, qs + js:qs + js + jw],
                        start=(kt == 0), stop=(kt == 1),
                    )
                ob = po.tile([128, jw], F32, tag="ob", name=f"ob{qs}_{m8}_{js}")
                nc.vector.tensor_copy(ob[:], op[:])
                nc.sync.dma_start(out_d[m8 * 128:(m8 + 1) * 128, qs + js:qs + js + jw], ob[:])

            # ---- software-pipelined attention ----
            # PE stream per head interleaves: this head's score matmuls, the
            # previous head's att@v accumulation, and filler work (V / next-Q
            # projections, previous block's O chunks) so PE never idles while
            # ACT streams the exps.
            def y_mt(st, mt):
                h, qs, qw, p_tiles, yps = st
                for ji, (js, jw) in enumerate(_chunks(qw, 512)):
                    nc.tensor.matmul(
                        yps[ji],
                        va_t[mt][:, h, :],
                        p_tiles[mt][:, js:js + jw],
                        start=(mt == 0), stop=(mt == MT - 1),
                    )

            def finish_head(st):
                h, qs, qw, p_tiles, yps = st
                pt, off = h // 2, (h % 2) * 64
                for ji, (js, jw) in enumerate(_chunks(qw, 512)):
                    yp = yps[ji]
                    rt = psm.tile([1, jw], F32, tag="rrow", name=f"rt{qs}_{h}_{js}")
                    nc.vector.tensor_copy(rt[:], yp[64:65, :])
                    rc = psm.tile([1, jw], F32, tag="recip", name=f"rc{qs}_{h}_{js}")
                    nc.vector.reciprocal_approx_fast(rc[:], rt[:])
                    rb = psm.tile([64, jw], F32, tag="rb", name=f"rb{qs}_{h}_{js}")
                    nc.gpsimd.partition_broadcast(rb[:], rc[:])
                    zsl = z_t[pt][off:off + 64, qs + js:qs + js + jw]
                    nc.vector.tensor_mul(zsl, yp[0:64, :], rb[:])
                    nc.vector.tensor_scalar_add(zsl, zsl, bv_t[pt][off:off + 64, :])

            prev = None      # in-flight head state awaiting y accumulation
            fillers = []     # list of zero-arg emitters, each ~one PE chain

            for mt in range(MT):
                fillers.append(lambda mt=mt: v_chain(mt))
            for kt in range(2):
                for js, jw in _chunks(1024, 512):
                    fillers.append(lambda kt=kt, js=js, jw=jw: q_chain(1024, kt, js, jw))

            fi = 0

            def pop_filler():
                nonlocal fi
                if fi < len(fillers):
                    fillers[fi]()
                    fi += 1

            for qi, (qs, qw) in enumerate(qblocks):
                for h in range(HPC):
                    pt, off = h // 2, (h % 2) * 64
                    p_tiles = []
                    yps = None
                    if prev is not None:
                        yps_prev = [
                            pb.tile([65, jw], F32, tag="y", name=f"y{prev[1]}_{prev[0]}_{js}")
                            for js, jw in _chunks(prev[2], 512)
                        ]
                        prev = (prev[0], prev[1], prev[2], prev[3], yps_prev)
                    for mt, (ms, mw) in enumerate(mts):
                        sp = pa.tile([mw, qw], F32, tag="wide", name=f"s{qs}_{h}_{mt}")
                        for js, jw in _chunks(qw, 512):
                            nc.tensor.matmul(
                                sp[:, js:js + jw],
                                k_t[pt][off:off + 64, ms:ms + mw],
                                q_t[pt][off:off + 64, qs + js:qs + js + jw],
                                start=True, stop=True,
                            )
                        px = pp.tile([mw, qw], BF16, tag="p", name=f"p{qs}_{h}_{mt}")
                        nc.scalar.activation(px[:], sp[:], EXP)
                        p_tiles.append(px)
                        if prev is not None:
                            y_mt(prev, mt)
                        pop_filler()
                    if prev is not None:
                        finish_head(prev)
                        if prev[0] == HPC - 1:
                            # previous block fully normalized: its O-projection
                            # chunks become filler work
                            pqs, pqw = prev[1], prev[2]
                            for m8 in range(8):
                                for js, jw in _chunks(pqw, 512):
                                    fillers.append(
                                        lambda pqs=pqs, m8=m8, js=js, jw=jw: o_chunk(pqs, m8, js, jw)
                                    )
                    prev = (h, qs, qw, p_tiles, None)

            # drain: final head's y accumulation + normalize, remaining fillers
            yps_prev = [
                pb.tile([65, jw], F32, tag="y", name=f"yfin_{js}")
                for js, jw in _chunks(prev[2], 512)
            ]
            prev = (prev[0], prev[1], prev[2], prev[3], yps_prev)
            for mt in range(MT):
                y_mt(prev, mt)
                pop_filler()
            finish_head(prev)
            while fi < len(fillers):
                pop_filler()
            qs, qw = prev[1], prev[2]
            for m8 in range(8):
                for js, jw in _chunks(qw, 512):
                    o_chunk(qs, m8, js, jw)

    nc.compile()
    return nc


_NC_CACHE = {}


def _get_nc(L_c):
    if L_c not in _NC_CACHE:
        _NC_CACHE[L_c] = _build(L_c)
    return _NC_CACHE[L_c]


def _install_ntff_hook():
    """Synthesize antenv.axon_hooks (missing in this image) so trace=True works."""
    import types

    if "antenv.axon_hooks" in sys.modules:
        return
    try:
        if "/root/.axon_site" not in sys.path:
            sys.path.insert(0, "/root/.axon_site")
        from trn_agent_boot.trn_boot import _ntff_profile_via_ctypes

        hook = _ntff_profile_via_ctypes("/opt/axon/libaxon_pjrt.so")
        mod = types.ModuleType("antenv.axon_hooks")
        mod.get_axon_ntff_profile_hook = lambda: hook
        import antenv  # noqa: F401

        sys.modules["antenv.axon_hooks"] = mod
    except Exception:
        pass


def kernel(query, att_mask, Wq, bq, Wk, bk, Wv, bv, Wo, bo):
    global LAST_EXEC_NS, LAST_RESULTS
    query = np.asarray(query, dtype=np.float32)
    mask = np.asarray(att_mask).astype(bool).reshape(B, L)
    Wq, bq = np.asarray(Wq, np.float32), np.asarray(bq, np.float32)
    Wk, bk = np.asarray(Wk, np.float32), np.asarray(bk, np.float32)
    Wv, bv = np.asarray(Wv, np.float32), np.asarray(bv, np.float32)
    Wo, bo = np.asarray(Wo, np.float32), np.asarray(bo, np.float32)

    valid = [np.nonzero(mask[b])[0] for b in range(B)]
    L_c = max(len(v) for v in valid)
    out = np.empty((B, D, L), np.float32)
    if L_c == 0:
        out[:] = bo[None, :, None]
        return out

    scale = np.float32(1.0 / np.sqrt(DK))
    # per-batch compacted keys + validity column
    xk_b, vones_b, xb_b = [], [], []
    for b in range(B):
        idx = valid[b]
        xk = np.zeros((D, L_c), np.float32)
        xk[:, :len(idx)] = query[b][:, idx]
        xk_b.append(xk.astype(NPBF16))
        vo = np.zeros((L_c, HPC, 1), np.float32)
        vo[:len(idx)] = 1.0
        vones_b.append(vo)
        xb_b.append(query[b].astype(NPBF16))

    in_maps = []
    for c in range(NCORES):
        b, g = divmod(c, NCORES // B)
        sl = slice(g * DH, (g + 1) * DH)
        in_maps.append({
            "xb": xb_b[b],
            "xk": xk_b[b],
            "vones": vones_b[b],
            "wq": np.ascontiguousarray((Wq[sl, :] * scale).T).astype(NPBF16),
            "wk": np.ascontiguousarray(Wk[sl, :].T).astype(NPBF16),
            "wv": np.ascontiguousarray(Wv[sl, :].T).astype(NPBF16),
            "wo": np.ascontiguousarray(Wo[:, sl].T).astype(NPBF16),
            "bq": (bq[sl] * scale).reshape(2, 128, 1).astype(np.float32),
            "bk": bk[sl].reshape(2, 128, 1).astype(np.float32),
            "bv": bv[sl].reshape(2, 128, 1).astype(np.float32),
        })

    nc = _get_nc(L_c)
    if TRACE:
        _install_ntff_hook()
    res = run_bass_kernel_spmd(nc, in_maps, core_ids=list(range(NCORES)), trace=TRACE)
    LAST_EXEC_NS = res.exec_time_ns
    LAST_RESULTS = res

    parts = [res.results[c]["out"] for c in range(NCORES)]
    for b in range(B):
        if len(valid[b]) == 0:
            out[b] = bo[:, None]
        else:
            acc = parts[4 * b].astype(np.float32)
            for g in range(1, 4):
                acc = acc + parts[4 * b + g]
            out[b] = acc + bo[:, None]
    return out
